# revision 8
# baseline (speedup 1.0000x reference)
"""Additive (Bahdanau) attention kernel for Trainium2, 8 NeuronCores.

Problem: B=4, H=16, L=8192, D=64 (fp32)
    e1 = q @ Wa_w.T + Wa_b ; e2 = k @ Ua_w.T + Ua_b
    t  = tanh(e1 + e2)
    e  = t @ va_w[0] + va_b          (va_b dropped: softmax shift-invariant)
    e  = where(mask == 0, -1e4, e)   (implemented as exp(e) * mask)
    alpha = softmax(e, axis=-1)      (over L)
    out = alpha[..., None] * v

Sharding: 64 independent (b, h) slices -> 8 per core, no collectives.

Per-slice layout: l = p * (L/128) + j with p = SBUF partition, j = tile
column; q/k/v/out live as [128, J, 64] (16KB contiguous per partition).
j-tiles are processed in PAIRS: transpose q[:, j0:j0+2, :] ([128,128]) on
TensorE fp32-transpose into psum rows (jj,d), then two K=128 matmuls with
block-diagonal [WaT|WaT], [UaT|UaT] bf16 weights accumulate e1+e2 for both
tiles as psum rows (jj,e).  tanh+bias on ScalarE -> bf16 tT2; one matmul
with tT2 stationary and a 2-column block-va moving operand yields both
score columns in natural [128, 1] layout.  Softmax sum uses a ones-matmul
(cross-partition reduce + broadcast in one), v-scale is a free-dim
broadcast multiply on VectorE.
"""

import numpy as np
from contextlib import ExitStack

import concourse.bass as bass
import concourse.tile as tile
from concourse import bacc, mybir
from concourse.bass_utils import run_bass_kernel_spmd

B, H, L, D = 4, 16, 8192, 64
N_CORES = 8
SLICES_PER_CORE = (B * H) // N_CORES
P = 128

F32 = mybir.dt.float32
BF16 = mybir.dt.bfloat16
I32 = mybir.dt.int32


def build_bass(n_slices=SLICES_PER_CORE, seq=L):
    JT = seq // P            # j-columns per slice
    NP = JT // 2             # tile pairs per slice
    assert JT % 2 == 0

    nc = bacc.Bacc(target_bir_lowering=False)
    q_ext = nc.declare_dram_parameter("q", [n_slices, seq, D], F32, isOutput=False)
    k_ext = nc.declare_dram_parameter("k", [n_slices, seq, D], F32, isOutput=False)
    v_ext = nc.declare_dram_parameter("v", [n_slices, seq, D], F32, isOutput=False)
    m_ext = nc.declare_dram_parameter("mask", [n_slices, seq], I32, isOutput=False)
    wb_ext = nc.declare_dram_parameter("wblk", [P, P], F32, isOutput=False)
    ub_ext = nc.declare_dram_parameter("ublk", [P, P], F32, isOutput=False)
    b2_ext = nc.declare_dram_parameter("bias2", [P, 1], F32, isOutput=False)
    va_ext = nc.declare_dram_parameter("vablk", [P, 2], F32, isOutput=False)
    id_ext = nc.declare_dram_parameter("ident", [P, P], F32, isOutput=False)
    on_ext = nc.declare_dram_parameter("ones", [P, P], F32, isOutput=False)
    out_ext = nc.declare_dram_parameter("out", [n_slices, seq, D], F32, isOutput=True)

    with tile.TileContext(nc) as tc, ExitStack() as ctx:
        consts = ctx.enter_context(tc.tile_pool(name="consts", bufs=1))
        bigq = ctx.enter_context(tc.tile_pool(name="bigq", bufs=2))
        bigk = ctx.enter_context(tc.tile_pool(name="bigk", bufs=2))
        bigv = ctx.enter_context(tc.tile_pool(name="bigv", bufs=2))
        bigo = ctx.enter_context(tc.tile_pool(name="bigo", bufs=2))
        chunks = ctx.enter_context(tc.tile_pool(name="chunks", bufs=3))
        smalls = ctx.enter_context(tc.tile_pool(name="smalls", bufs=2))
        ps_t = ctx.enter_context(tc.tile_pool(name="ps_t", bufs=3, space="PSUM"))
        ps_e = ctx.enter_context(tc.tile_pool(name="ps_e", bufs=2, space="PSUM"))
        ps_sc = ctx.enter_context(tc.tile_pool(name="ps_sc", bufs=2, space="PSUM"))
        ps_s = ctx.enter_context(tc.tile_pool(name="ps_s", bufs=1, space="PSUM"))

        # constants (loaded once; SWDGE casts f32 -> bf16 in the DMA)
        wblk = consts.tile([P, P], BF16)
        nc.gpsimd.dma_start(wblk[:], wb_ext[:, :])
        ublk = consts.tile([P, P], BF16)
        nc.gpsimd.dma_start(ublk[:], ub_ext[:, :])
        vablk = consts.tile([P, 2], BF16)
        nc.gpsimd.dma_start(vablk[:], va_ext[:, :])
        bias2 = consts.tile([P, 1], F32)
        nc.sync.dma_start(bias2[:], b2_ext[:, :])
        ident = consts.tile([P, P], BF16)
        nc.gpsimd.dma_start(ident[:], id_ext[:, :])
        ones = consts.tile([P, P], F32)
        nc.sync.dma_start(ones[:], on_ext[:, :])

        for s in range(n_slices):
            q_sb = bigq.tile([P, JT, D], BF16)
            nc.gpsimd.dma_start(q_sb[:], q_ext[s].rearrange("(p j) d -> p j d", p=P))
            k_sb = bigk.tile([P, JT, D], BF16)
            nc.gpsimd.dma_start(k_sb[:], k_ext[s].rearrange("(p j) d -> p j d", p=P))
            mask_sb = smalls.tile([P, JT], I32)
            nc.sync.dma_start(mask_sb[:], m_ext[s].rearrange("(p j) -> p j", p=P))

            scores_ps = ps_sc.tile([P, JT], F32)

            for u in range(NP):
                j0 = 2 * u
                pQK = ps_t.tile([P, 2 * P], F32)
                nc.tensor.matmul(pQK[:, 0:P],
                                 q_sb[:, j0:j0 + 2, :].rearrange("p a d -> p (a d)"),
                                 ident[:], start=True, stop=True)
                nc.tensor.matmul(pQK[:, P:2 * P],
                                 k_sb[:, j0:j0 + 2, :].rearrange("p a d -> p (a d)"),
                                 ident[:], start=True, stop=True)
                qkT2 = chunks.tile([P, 2 * P], BF16, tag="qkT2")
                nc.vector.tensor_copy(qkT2[:], pQK[:])
                pE2 = ps_e.tile([P, P], F32)
                nc.tensor.matmul(pE2[:], wblk[:], qkT2[:, 0:P],
                                 start=True, stop=False)
                nc.tensor.matmul(pE2[:], ublk[:], qkT2[:, P:2 * P],
                                 start=False, stop=True)
                tT2 = chunks.tile([P, P], BF16, tag="tT2")
                nc.scalar.activation(tT2[:], pE2[:],
                                     mybir.ActivationFunctionType.Tanh,
                                     bias=bias2[:], scale=1.0)
                nc.tensor.matmul(scores_ps[:, j0:j0 + 2], tT2[:], vablk[:],
                                 start=True, stop=True)

            # softmax weights: alpha = exp(e) * mask / sum(exp(e) * mask)
            maskf = smalls.tile([P, JT], F32)
            nc.vector.tensor_copy(maskf[:], mask_sb[:])
            p_sb = smalls.tile([P, JT], F32)
            nc.scalar.activation(p_sb[:], scores_ps[:],
                                 mybir.ActivationFunctionType.Exp)
            nc.vector.tensor_mul(p_sb[:], p_sb[:], maskf[:])
            rowsum = smalls.tile([P, 1], F32)
            nc.vector.tensor_reduce(rowsum[:], p_sb[:],
                                    axis=mybir.AxisListType.X,
                                    op=mybir.AluOpType.add)
            sum_ps = ps_s.tile([P, 1], F32)
            nc.tensor.matmul(sum_ps[:], ones[:], rowsum[:], start=True, stop=True)
            invs = smalls.tile([P, 1], F32)
            nc.vector.reciprocal(invs[:], sum_ps[:])
            alpha = smalls.tile([P, JT], F32)
            nc.vector.tensor_scalar_mul(alpha[:], p_sb[:], invs[:])

            v_sb = bigv.tile([P, JT, D], F32)
            nc.sync.dma_start(v_sb[:], v_ext[s].rearrange("(p j) d -> p j d", p=P))
            o_sb = bigo.tile([P, JT, D], F32)
            nc.vector.tensor_mul(o_sb[:], v_sb[:],
                                 alpha[:, :, None].to_broadcast([P, JT, D]))
            nc.sync.dma_start(out_ext[s].rearrange("(p j) d -> p j d", p=P), o_sb[:])

    nc.compile()
    return nc


def make_host_inputs(q, k, v, mask, Wa_w, Wa_b, Ua_w, Ua_b, va_w):
    """Returns per-core in_maps for the full problem."""
    q = np.ascontiguousarray(np.asarray(q, np.float32).reshape(B * H, L, D))
    k = np.ascontiguousarray(np.asarray(k, np.float32).reshape(B * H, L, D))
    v = np.ascontiguousarray(np.asarray(v, np.float32).reshape(B * H, L, D))
    mask = np.ascontiguousarray(np.asarray(mask, np.int32).reshape(B * H, L))

    WaT = np.asarray(Wa_w, np.float32).T  # [d, e]
    UaT = np.asarray(Ua_w, np.float32).T
    wblk = np.zeros((P, P), np.float32)
    wblk[0:D, 0:D] = WaT
    wblk[D:2 * D, D:2 * D] = WaT
    ublk = np.zeros((P, P), np.float32)
    ublk[0:D, 0:D] = UaT
    ublk[D:2 * D, D:2 * D] = UaT
    be = (np.asarray(Wa_b, np.float32) + np.asarray(Ua_b, np.float32))
    bias2 = np.concatenate([be, be]).reshape(P, 1)
    va = np.asarray(va_w, np.float32)[0]
    vablk = np.zeros((P, 2), np.float32)
    vablk[0:D, 0] = va
    vablk[D:2 * D, 1] = va
    ident = np.eye(P, dtype=np.float32)
    ones = np.ones((P, P), dtype=np.float32)

    in_maps = []
    for i in range(N_CORES):
        sl = slice(i * SLICES_PER_CORE, (i + 1) * SLICES_PER_CORE)
        in_maps.append({
            "q": q[sl], "k": k[sl], "v": v[sl], "mask": mask[sl],
            "wblk": wblk, "ublk": ublk, "bias2": bias2, "vablk": vablk,
            "ident": ident, "ones": ones,
        })
    return in_maps


_CACHED_NC = None


def kernel(q, k, v, mask, Wa_w, Wa_b, Ua_w, Ua_b, va_w, va_b=None, **kwargs):
    global _CACHED_NC
    if _CACHED_NC is None:
        _CACHED_NC = build_bass()
    in_maps = make_host_inputs(q, k, v, mask, Wa_w, Wa_b, Ua_w, Ua_b, va_w)
    res = run_bass_kernel_spmd(_CACHED_NC, in_maps, list(range(N_CORES)))
    out = np.concatenate([r["out"] for r in res.results], axis=0)
    return np.ascontiguousarray(out.reshape(B, H, L, D).astype(np.float32))


# revision 11
# speedup vs baseline: 1.1811x; 1.1811x over previous
"""Additive (Bahdanau) attention kernel for Trainium2, 8 NeuronCores.

Problem: B=4, H=16, L=8192, D=64 (fp32)
    e1 = q @ Wa_w.T + Wa_b ; e2 = k @ Ua_w.T + Ua_b
    t  = tanh(e1 + e2)
    e  = t @ va_w[0] + va_b          (va_b dropped: softmax shift-invariant)
    e  = where(mask == 0, -1e4, e)   (implemented as exp(e) * mask)
    alpha = softmax(e, axis=-1)      (over L)
    out = alpha[..., None] * v

Sharding: 64 independent (b, h) slices -> 8 per core, no collectives.

Per-slice layout: l = p * (L/128) + j with p = SBUF partition, j = tile
column; q/k/v live as [128, J, 64] (contiguous per partition), q/k are
cast to bf16 by the SWDGE DMA.  j-tiles are processed in PAIRS (transpose
q[:, j0:j0+2, :] [128,128] on TensorE via identity matmul -> psum rows
(jj,d)), pairs are processed in GROUPS of 4 to batch the weight matmuls
(N=512) and tanh.  Block-diagonal [WaT|WaT], [UaT|UaT] bf16 weights give
e1+e2 for both tiles of a pair in one K=128 contraction; tanh+bias on
ScalarE -> bf16; per-pair score matmul with tT stationary and a 2-column
block-va moving operand yields both score columns in natural [128, 1]
layout.  Softmax sum uses a ones-matmul (cross-partition reduce +
broadcast in one), v-scale is a free-dim broadcast multiply on GpSimd,
output is written bf16 (upcast on host).
"""

import numpy as np
from contextlib import ExitStack

import concourse.bass as bass
import concourse.tile as tile
from concourse import bacc, mybir
from concourse.bass_utils import run_bass_kernel_spmd

B, H, L, D = 4, 16, 8192, 64
N_CORES = 8
SLICES_PER_CORE = (B * H) // N_CORES
P = 128

F32 = mybir.dt.float32
BF16 = mybir.dt.bfloat16
I32 = mybir.dt.int32


def build_bass(n_slices=SLICES_PER_CORE, seq=L):
    JT = seq // P            # j-columns per slice
    NPAIR = JT // 2          # tile pairs per slice
    NGRP = NPAIR // 4        # groups of 4 pairs
    assert NPAIR % 4 == 0

    nc = bacc.Bacc(target_bir_lowering=False)
    q_ext = nc.declare_dram_parameter("q", [n_slices, seq, D], F32, isOutput=False)
    k_ext = nc.declare_dram_parameter("k", [n_slices, seq, D], F32, isOutput=False)
    v_ext = nc.declare_dram_parameter("v", [n_slices, seq, D], F32, isOutput=False)
    m_ext = nc.declare_dram_parameter("mask", [n_slices, seq], I32, isOutput=False)
    wb_ext = nc.declare_dram_parameter("wblk", [P, P], F32, isOutput=False)
    ub_ext = nc.declare_dram_parameter("ublk", [P, P], F32, isOutput=False)
    b2_ext = nc.declare_dram_parameter("bias2", [P, 1], F32, isOutput=False)
    va_ext = nc.declare_dram_parameter("vablk", [P, 2], F32, isOutput=False)
    id_ext = nc.declare_dram_parameter("ident", [P, P], F32, isOutput=False)
    on_ext = nc.declare_dram_parameter("ones", [P, P], F32, isOutput=False)
    out_ext = nc.declare_dram_parameter("out", [n_slices, seq, D], BF16,
                                        isOutput=True)

    with tile.TileContext(nc) as tc, ExitStack() as ctx:
        consts = ctx.enter_context(tc.tile_pool(name="consts", bufs=1))
        bigq = ctx.enter_context(tc.tile_pool(name="bigq", bufs=2))
        bigk = ctx.enter_context(tc.tile_pool(name="bigk", bufs=2))
        bigv = ctx.enter_context(tc.tile_pool(name="bigv", bufs=2))
        bigo = ctx.enter_context(tc.tile_pool(name="bigo", bufs=2))
        chunks = ctx.enter_context(tc.tile_pool(name="chunks", bufs=3))
        smalls = ctx.enter_context(tc.tile_pool(name="smalls", bufs=2))
        ps_t = ctx.enter_context(tc.tile_pool(name="ps_t", bufs=3, space="PSUM"))
        ps_e = ctx.enter_context(tc.tile_pool(name="ps_e", bufs=2, space="PSUM"))
        ps_sc = ctx.enter_context(tc.tile_pool(name="ps_sc", bufs=2, space="PSUM"))
        ps_s = ctx.enter_context(tc.tile_pool(name="ps_s", bufs=1, space="PSUM"))

        # constants (loaded once; SWDGE casts f32 -> bf16 in the DMA)
        wblk = consts.tile([P, P], BF16)
        nc.gpsimd.dma_start(wblk[:], wb_ext[:, :])
        ublk = consts.tile([P, P], BF16)
        nc.gpsimd.dma_start(ublk[:], ub_ext[:, :])
        vablk = consts.tile([P, 2], BF16)
        nc.gpsimd.dma_start(vablk[:], va_ext[:, :])
        bias2 = consts.tile([P, 1], F32)
        nc.sync.dma_start(bias2[:], b2_ext[:, :])
        ident = consts.tile([P, P], BF16)
        nc.gpsimd.dma_start(ident[:], id_ext[:, :])
        ones = consts.tile([P, P], F32)
        nc.sync.dma_start(ones[:], on_ext[:, :])

        for s in range(n_slices):
            q_sb = bigq.tile([P, JT, D], BF16)
            nc.gpsimd.dma_start(q_sb[:], q_ext[s].rearrange("(p j) d -> p j d", p=P))
            k_sb = bigk.tile([P, JT, D], BF16)
            nc.gpsimd.dma_start(k_sb[:], k_ext[s].rearrange("(p j) d -> p j d", p=P))
            mask_sb = smalls.tile([P, JT], I32)
            nc.sync.dma_start(mask_sb[:], m_ext[s].rearrange("(p j) -> p j", p=P))

            scores_ps = ps_sc.tile([P, JT], F32)

            for g in range(NGRP):
                # [128, pair, 256]: per pair cols 0:128 = qT2, 128:256 = kT2
                qkT4 = chunks.tile([P, 4, 2 * P], BF16, tag="qkT4")
                for h in range(2):          # two 2-pair transpose blocks
                    pQK2 = ps_t.tile([P, 4 * P], F32)
                    for b in range(2):      # pair within block
                        u = g * 4 + 2 * h + b
                        j0 = 2 * u
                        nc.tensor.matmul(
                            pQK2[:, 2 * b * P:(2 * b + 1) * P],
                            q_sb[:, j0:j0 + 2, :].rearrange("p a d -> p (a d)"),
                            ident[:], start=True, stop=True)
                        nc.tensor.matmul(
                            pQK2[:, (2 * b + 1) * P:(2 * b + 2) * P],
                            k_sb[:, j0:j0 + 2, :].rearrange("p a d -> p (a d)"),
                            ident[:], start=True, stop=True)
                    nc.vector.tensor_copy(
                        qkT4[:, 2 * h:2 * h + 2, :].rearrange("p a c -> p (a c)"),
                        pQK2[:])
                pE4 = ps_e.tile([P, 4 * P], F32)
                nc.tensor.matmul(pE4.rearrange("p (a c) -> p a c", a=4),
                                 wblk[:], qkT4[:, :, 0:P],
                                 start=True, stop=False)
                nc.tensor.matmul(pE4.rearrange("p (a c) -> p a c", a=4),
                                 ublk[:], qkT4[:, :, P:2 * P],
                                 start=False, stop=True)
                tT4 = chunks.tile([P, 4 * P], BF16, tag="tT4")
                nc.scalar.activation(tT4[:], pE4[:],
                                     mybir.ActivationFunctionType.Tanh,
                                     bias=bias2[:], scale=1.0)
                for pr in range(4):
                    j0 = 2 * (g * 4 + pr)
                    nc.tensor.matmul(scores_ps[:, j0:j0 + 2],
                                     tT4[:, pr * P:(pr + 1) * P], vablk[:],
                                     start=True, stop=True)

            # softmax weights: alpha = exp(e) * mask / sum(exp(e) * mask)
            maskf = smalls.tile([P, JT], F32)
            nc.vector.tensor_copy(maskf[:], mask_sb[:])
            p_sb = smalls.tile([P, JT], F32)
            nc.scalar.activation(p_sb[:], scores_ps[:],
                                 mybir.ActivationFunctionType.Exp)
            nc.vector.tensor_mul(p_sb[:], p_sb[:], maskf[:])
            rowsum = smalls.tile([P, 1], F32)
            nc.vector.tensor_reduce(rowsum[:], p_sb[:],
                                    axis=mybir.AxisListType.X,
                                    op=mybir.AluOpType.add)
            sum_ps = ps_s.tile([P, 1], F32)
            nc.tensor.matmul(sum_ps[:], ones[:], rowsum[:], start=True, stop=True)
            invs = smalls.tile([P, 1], F32)
            nc.vector.reciprocal(invs[:], sum_ps[:])
            alpha = smalls.tile([P, JT], F32)
            nc.vector.tensor_scalar_mul(alpha[:], p_sb[:], invs[:])

            v_sb = bigv.tile([P, JT, D], F32)
            nc.sync.dma_start(v_sb[:], v_ext[s].rearrange("(p j) d -> p j d", p=P))
            o_sb = bigo.tile([P, JT, D], BF16)
            nc.gpsimd.tensor_tensor(o_sb[:], v_sb[:],
                                    alpha[:, :, None].to_broadcast([P, JT, D]),
                                    op=mybir.AluOpType.mult)
            nc.sync.dma_start(out_ext[s].rearrange("(p j) d -> p j d", p=P), o_sb[:])

    nc.compile()
    return nc


def make_host_inputs(q, k, v, mask, Wa_w, Wa_b, Ua_w, Ua_b, va_w):
    """Returns per-core in_maps for the full problem."""
    q = np.ascontiguousarray(np.asarray(q, np.float32).reshape(B * H, L, D))
    k = np.ascontiguousarray(np.asarray(k, np.float32).reshape(B * H, L, D))
    v = np.ascontiguousarray(np.asarray(v, np.float32).reshape(B * H, L, D))
    mask = np.ascontiguousarray(np.asarray(mask, np.int32).reshape(B * H, L))

    WaT = np.asarray(Wa_w, np.float32).T  # [d, e]
    UaT = np.asarray(Ua_w, np.float32).T
    wblk = np.zeros((P, P), np.float32)
    wblk[0:D, 0:D] = WaT
    wblk[D:2 * D, D:2 * D] = WaT
    ublk = np.zeros((P, P), np.float32)
    ublk[0:D, 0:D] = UaT
    ublk[D:2 * D, D:2 * D] = UaT
    be = (np.asarray(Wa_b, np.float32) + np.asarray(Ua_b, np.float32))
    bias2 = np.concatenate([be, be]).reshape(P, 1)
    va = np.asarray(va_w, np.float32)[0]
    vablk = np.zeros((P, 2), np.float32)
    vablk[0:D, 0] = va
    vablk[D:2 * D, 1] = va
    ident = np.eye(P, dtype=np.float32)
    ones = np.ones((P, P), dtype=np.float32)

    in_maps = []
    for i in range(N_CORES):
        sl = slice(i * SLICES_PER_CORE, (i + 1) * SLICES_PER_CORE)
        in_maps.append({
            "q": q[sl], "k": k[sl], "v": v[sl], "mask": mask[sl],
            "wblk": wblk, "ublk": ublk, "bias2": bias2, "vablk": vablk,
            "ident": ident, "ones": ones,
        })
    return in_maps


_CACHED_NC = None


def kernel(q, k, v, mask, Wa_w, Wa_b, Ua_w, Ua_b, va_w, va_b=None, **kwargs):
    global _CACHED_NC
    if _CACHED_NC is None:
        _CACHED_NC = build_bass()
    in_maps = make_host_inputs(q, k, v, mask, Wa_w, Wa_b, Ua_w, Ua_b, va_w)
    res = run_bass_kernel_spmd(_CACHED_NC, in_maps, list(range(N_CORES)))
    out = np.concatenate([np.asarray(r["out"], np.float32) for r in res.results],
                         axis=0)
    return np.ascontiguousarray(out.reshape(B, H, L, D).astype(np.float32))
